# revision 12
# baseline (speedup 1.0000x reference)
"""GATv2 attention-pool kernel for 8 Trainium2 NeuronCores.

Algorithm
---------
Reference computes, per edge e with target node t(e):
    feats = q + k                                   [E, 64]
    logits[e,h] = sum_c feats[e,h*8+c] * A[c,h]     [E, 8]
    attn = segment_softmax(logits, targets)         [E, 8]
    out[n] = relu(segment_sum(q * attn))            [N, 64]

Because logits are O(20), exp() never overflows fp32/bf16, so the
segment-max shift is unnecessary and softmax folds into two segment-SUMS:
    denom[n,h]  = sum_{e->n} exp(logits[e,h])
    pooled[n,:] = sum_{e->n} q[e,:] * exp(logits[e,h])
    out[n]      = relu(pooled[n] / denom[n])

Distribution: edges are partitioned by target node (host-side sort), 100000
nodes split into 8 contiguous shards of 12500 -> all segment reductions are
core-local, no collectives.  Each shard is cut into windows of <= 32 nodes
whose edges are padded to TSUB*128 slots; G windows form a group brought in
by one contiguous DMA (12KB/partition descriptors).

Datapath is fp16 (bf16 for exp outputs, f32 psum accumulation):
 - q/k columns are host-permuted to (c,h) order so the per-head tree
   reduction and the ex-broadcast multiply keep a packed innermost dim.
 - the one-hot selector S is stored [node, subtile] so the rel broadcast
   lands on a middle dim; one tensor_tensor(is_equal) builds it per group.
 - per 128-edge subtile the PE accumulates psum += S^T @ [q*ex | ex]
   in fp16/bf16 (4x faster than fp32 matmul); two 32-node windows stack
   into the 64 psum partitions, then the epilogue divides, relus and
   permutes back to (h,c) once per node.

Host work is index metadata + data layout only (argsort of targets, gather
of q/k rows into slot order, fp16 packing); all floating-point arithmetic
runs on device.
"""

import os
import sys

import numpy as np

N_NODES = 100000
N_EDGES = 1600000
H = 8
C = 8
HC = H * C
MW = 2 * HC
N_CORES = 8
NODES_PER_CORE = N_NODES // N_CORES
WIN_NODES = 32
SUB = 128
TSUB = 4          # subtiles per window (window edge capacity = TSUB*128)
G = 12            # windows per group (one DMA + one DVE pass per group)
PJ = HC + H       # psum cols per window: 64 numerator + 8 denominator


def _ensure_imports():
    try:
        import concourse.bass  # noqa: F401
    except ImportError:
        for p in ("/opt/trn_rl_repo", "/root/.axon_site/_ro/trn_rl_repo"):
            if os.path.isdir(p) and p not in sys.path:
                sys.path.insert(0, p)


def pack_windows(targets):
    """Sort edges by target; bin-pack each core's nodes into windows.

    Every window holds at most WIN_NODES nodes AND at most TSUB*SUB edges
    (two-pointer big+small pairing keeps fragmentation low).
    """
    order = np.argsort(targets, kind="stable")
    tsorted = targets[order]
    node_start = np.searchsorted(tsorted, np.arange(N_NODES + 1))
    deg = np.diff(node_start)

    cap_e = TSUB * SUB
    packs = []
    for c in range(N_CORES):
        nodes = np.arange(c * NODES_PER_CORE, (c + 1) * NODES_PER_CORE)
        by_deg = nodes[np.argsort(-deg[nodes], kind="stable")]
        # best-fit decreasing: place each node into the open window with
        # the least remaining edge capacity that still fits it
        rem_e = np.empty(0, dtype=np.int64)
        rem_n = np.empty(0, dtype=np.int64)
        wins = []
        for node in by_deg:
            d = int(deg[node])
            cand = np.where((rem_e >= d) & (rem_n > 0))[0]
            if len(cand):
                wi = cand[np.argmin(rem_e[cand])]
            else:
                wi = len(wins)
                wins.append([])
                rem_e = np.append(rem_e, cap_e)
                rem_n = np.append(rem_n, WIN_NODES)
            wins[wi].append(node)
            rem_e[wi] -= d
            rem_n[wi] -= 1
        packs.append(wins)
    return packs, order, node_start


def build_slots(packs, order, node_start):
    """Lay out edge slots in DRAM row order r = gbase + p*Tg + t.

    Window wi of a group owns subtile columns [wi*TSUB, (wi+1)*TSUB); within
    a window, edge j sits at (p = j // TSUB, t_local = j % TSUB), so each
    node's slot run is contiguous through the group-slab view.
    """
    n_win = max(len(w) for w in packs)
    n_groups = (n_win + G - 1) // G
    group_sizes = [min(G, n_win - gi * G) for gi in range(n_groups)]
    n_slots = sum(128 * gs * TSUB for gs in group_sizes)

    perms = np.zeros((N_CORES, n_slots), dtype=np.int64)
    rels = np.full((N_CORES, n_slots), -1.0, dtype=np.float16)
    node_order = np.full((N_CORES, n_win * WIN_NODES), -1, dtype=np.int64)
    for c in range(N_CORES):
        gbase = 0
        for gi, gs in enumerate(group_sizes):
            tg = gs * TSUB
            pslab = perms[c, gbase:gbase + 128 * tg].reshape(128, tg)
            rslab = rels[c, gbase:gbase + 128 * tg].reshape(128, tg)
            for wl in range(gs):
                w = gi * G + wl
                if w >= len(packs[c]):
                    continue
                pw = pslab[:, wl * TSUB:(wl + 1) * TSUB]
                rw = rslab[:, wl * TSUB:(wl + 1) * TSUB]
                pos = 0
                for j, node in enumerate(packs[c][w]):
                    e0, e1 = node_start[node], node_start[node + 1]
                    cnt = e1 - e0
                    # .flat writes through the non-contiguous column view
                    pw.flat[pos:pos + cnt] = order[e0:e1]
                    rw.flat[pos:pos + cnt] = j
                    pos += cnt
                    node_order[c, w * WIN_NODES + j] = node
            gbase += 128 * tg
    return perms, rels, node_order, n_win, n_slots, group_sizes


def build_nc(n_slots, n_win, group_sizes):
    """Build the single SPMD Bass program for one core's shard."""
    _ensure_imports()
    import concourse.bacc as bacc
    import concourse.mybir as mybir
    import concourse.tile as tile

    f32 = mybir.dt.float32
    f16 = mybir.dt.float16
    bf16 = mybir.dt.bfloat16

    OP = mybir.AluOpType
    AF = mybir.ActivationFunctionType

    tgmax = G * TSUB
    # two 32-node windows stack into 64 psum partitions; nb = column blocks
    nbs = [(gs + 1) // 2 for gs in group_sizes]
    nb_total = sum(nbs)

    nc = bacc.Bacc("TRN2", num_devices=N_CORES)
    qk = nc.declare_dram_parameter("qk", [n_slots, MW], f16, False)
    rel = nc.declare_dram_parameter("rel", [n_slots], f16, False)
    wrow = nc.declare_dram_parameter("wrow", [128, HC], f16, False)
    iof = nc.declare_dram_parameter("iof", [128, WIN_NODES * tgmax], f16,
                                    False)
    out = nc.declare_dram_parameter("out", [2 * WIN_NODES, nb_total * HC],
                                    f32, isOutput=True)

    with tile.TileContext(nc) as tc:
        with (
            tc.tile_pool(name="const", bufs=1) as cpool,
            tc.tile_pool(name="qk", bufs=4) as qkpool,
            tc.tile_pool(name="mid", bufs=2) as midpool,
            tc.tile_pool(name="lgp", bufs=4) as lgpool,
            tc.tile_pool(name="mm", bufs=4) as mmpool,
            tc.tile_pool(name="mt", bufs=3) as mtpool,
            tc.tile_pool(name="fin", bufs=3) as finpool,
            tc.tile_pool(name="psum", bufs=8, space="PSUM") as ppool,
        ):
            w_t = cpool.tile([128, HC], f16)
            nc.sync.dma_start(out=w_t[:], in_=wrow[:])
            io_t = cpool.tile([128, WIN_NODES, tgmax], f16)
            nc.sync.dma_start(
                out=io_t[:],
                in_=iof[:].rearrange("p (n t) -> p n t", t=tgmax))

            def emit_load(gi, gbase, gs):
                tg = gs * TSUB
                nsl = 128 * tg
                qk_t = qkpool.tile([128, tg, MW], f16, tag="qk")
                nc.sync.dma_start(
                    out=qk_t[:],
                    in_=qk[gbase:gbase + nsl, :].rearrange(
                        "(p t) c -> p t c", p=128),
                )
                r_t = qkpool.tile([128, tg], f16, tag="r")
                nc.sync.dma_start(
                    out=r_t[:],
                    in_=rel[gbase:gbase + nsl].rearrange(
                        "(p t) -> p t", p=128),
                )
                return {"gi": gi, "gs": gs, "tg": tg, "qk": qk_t, "r": r_t}

            def emit_logits(s):
                tg = s["tg"]
                qk3 = s["qk"]
                # f = q + k   (tensor_tensor is the fastest DVE op on hw)
                f_t = midpool.tile([128, tg, HC], f16, tag="f")
                nc.vector.tensor_tensor(
                    out=f_t[:], in0=qk3[:, :, 0:HC],
                    in1=qk3[:, :, HC:MW], op=OP.add)
                # wf = f * w  (w broadcast over t: middle dim, stays packed)
                wf_t = midpool.tile([128, tg, HC], f16, tag="wf")
                nc.vector.tensor_tensor(
                    out=wf_t[:], in0=f_t[:],
                    in1=w_t[:, None, :].to_broadcast([128, tg, HC]),
                    op=OP.mult)
                # tree-reduce over c (c-major layout: head h at col c*8+h)
                t1 = midpool.tile([128, tg, HC // 2], f16, tag="t1")
                nc.vector.tensor_tensor(
                    out=t1[:], in0=wf_t[:, :, 0:32],
                    in1=wf_t[:, :, 32:64], op=OP.add)
                # the two small tree levels run on the otherwise-idle gpsimd
                t2 = midpool.tile([128, tg, HC // 4], f16, tag="t2")
                nc.gpsimd.tensor_tensor(
                    out=t2[:], in0=t1[:, :, 0:16],
                    in1=t1[:, :, 16:32], op=OP.add)
                lg = lgpool.tile([128, tg, H], f16, tag="lg")
                nc.gpsimd.tensor_tensor(
                    out=lg[:], in0=t2[:, :, 0:8],
                    in1=t2[:, :, 8:16], op=OP.add)
                s["lg"] = lg

            def emit_spath(s):
                tg = s["tg"]
                # S[p, n, t] = (iota[n] == rel[p, t]) : rel broadcast on the
                # middle dim keeps the packed last dim
                s_t = mmpool.tile([128, WIN_NODES, tg], f16, tag="S")
                nc.vector.tensor_tensor(
                    out=s_t[:], in0=io_t[:, :, 0:tg],
                    in1=s["r"][:, None, :].to_broadcast(
                        [128, WIN_NODES, tg]),
                    op=OP.is_equal)
                s["S"] = s_t

            def emit_m(s):
                tg = s["tg"]
                m_t = mtpool.tile([128, tg, PJ], bf16, tag="M")
                nc.scalar.activation(
                    out=m_t[:, :, HC:PJ], in_=s["lg"][:], func=AF.Exp)
                # m = q * ex (ex broadcast over c: middle dim, packed h last)
                nc.vector.tensor_tensor(
                    out=m_t[:, :, 0:HC].rearrange(
                        "p t (c h) -> p t c h", h=H),
                    in0=s["qk"][:, :, 0:HC].rearrange(
                        "p t (c h) -> p t c h", h=H),
                    in1=m_t[:, :, HC:PJ][:, :, None, :].to_broadcast(
                        [128, tg, C, H]),
                    op=OP.mult)
                s["m"] = m_t

            def emit_mm(s):
                gs = s["gs"]
                nb = (gs + 1) // 2
                p_t = ppool.tile([2 * WIN_NODES, nb * PJ], f32)
                for wl in range(gs):
                    x, b = wl % 2, wl // 2
                    prow = slice(x * WIN_NODES, (x + 1) * WIN_NODES)
                    pcols = slice(b * PJ, (b + 1) * PJ)
                    for g in range(TSUB):
                        t = wl * TSUB + g
                        nc.tensor.matmul(
                            p_t[prow, pcols],
                            lhsT=s["S"][:, :, t],
                            rhs=s["m"][:, t, :],
                            start=(g == 0),
                            stop=(g == TSUB - 1),
                        )
                s["psum"] = p_t

            def emit_epilogue(s, bbase):
                gs = s["gs"]
                nb = (gs + 1) // 2
                # drain psum through the (idle) scalar engine, folding the
                # (c,h) -> (h,c) permute into the numerator copy; the
                # vector ops then stream contiguous SBUF
                p3 = s["psum"][:].rearrange("p (w j) -> p w j", j=PJ)
                ppn = finpool.tile([2 * WIN_NODES, nb, H, C], f32,
                                   tag="ppn")
                nc.scalar.activation(
                    out=ppn[:].rearrange("p w h c -> p w c h"),
                    in_=p3[:, :, 0:HC].rearrange(
                        "p w (c h) -> p w c h", h=H),
                    func=AF.Copy)
                ppd = finpool.tile([2 * WIN_NODES, nb, H], f32, tag="ppd")
                nc.scalar.activation(
                    out=ppd[:], in_=p3[:, :, HC:PJ], func=AF.Copy)
                rc_t = finpool.tile([2 * WIN_NODES, nb, H], f32, tag="rc")
                nc.vector.reciprocal(rc_t[:], ppd[:])
                d_t = finpool.tile([2 * WIN_NODES, nb, H, C], f32, tag="d")
                nc.vector.tensor_tensor(
                    out=d_t[:],
                    in0=ppn[:],
                    in1=rc_t[:, :, :, None].to_broadcast(
                        [2 * WIN_NODES, nb, H, C]),
                    op=OP.mult)
                o_t = finpool.tile([2 * WIN_NODES, nb, HC], f32, tag="o")
                nc.scalar.activation(
                    o_t[:], d_t[:].rearrange("p w h c -> p w (h c)"),
                    func=AF.Relu)
                nc.sync.dma_start(
                    out=out[:, bbase * HC:(bbase + nb) * HC].rearrange(
                        "p (w c) -> p w c", c=HC),
                    in_=o_t[:],
                )

            gbases, bbases = [], []
            gb = bb = 0
            for gs, nb in zip(group_sizes, nbs):
                gbases.append(gb)
                bbases.append(bb)
                gb += 128 * gs * TSUB
                bb += nb

            ng = len(group_sizes)

            def load(i):
                return emit_load(i, gbases[i], group_sizes[i])

            # software pipeline with 2-group lookahead: the epilogue of
            # group i is emitted two iterations after its matmuls, so the
            # DVE always has group i+2's logits/S work while the PE
            # finishes group i+1
            st = [None] * ng
            st[0] = load(0)
            emit_logits(st[0])
            emit_spath(st[0])
            if ng > 1:
                st[1] = load(1)
            for i in range(ng):
                if i + 2 < ng:
                    st[i + 2] = load(i + 2)
                emit_m(st[i])
                emit_mm(st[i])
                if i + 1 < ng:
                    emit_logits(st[i + 1])
                    emit_spath(st[i + 1])
                if i >= 1:
                    emit_epilogue(st[i - 1], bbases[i - 1])
                    st[i - 1] = None
            emit_epilogue(st[ng - 1], bbases[ng - 1])

    nc.finalize()
    return nc


def _host_arrays(query, key, attn_kernel, targets):
    packs, order, node_start = pack_windows(targets)
    perms, rels, node_order, n_win, n_slots, group_sizes = build_slots(
        packs, order, node_start)

    # (c,h)-major column permutation: new col c*8+h <- orig col h*8+c
    colperm = (np.arange(HC).reshape(C, H).T).reshape(-1)  # [c*8+h] = h*8+c
    wrow_1 = np.asarray(attn_kernel, dtype=np.float16).reshape(-1)  # A[c,h]
    wrow = np.tile(wrow_1, (128, 1))

    tgmax = G * TSUB
    iof = np.tile(
        np.repeat(np.arange(WIN_NODES, dtype=np.float16), tgmax), (128, 1))

    q16 = query[:, colperm].astype(np.float16)
    k16 = key[:, colperm].astype(np.float16)
    in_maps = []
    for c in range(N_CORES):
        qkc = np.zeros((n_slots, MW), dtype=np.float16)
        pc = perms[c]
        used = rels[c] >= 0
        qkc[used, :HC] = q16[pc[used]]
        qkc[used, HC:] = k16[pc[used]]
        in_maps.append({
            "qk": qkc,
            "rel": rels[c],
            "wrow": wrow,
            "iof": iof,
        })
    return in_maps, node_order, n_win, n_slots, group_sizes


TRACE = False          # set by test harness to capture an NTFF profile
TRACE_CORES = None
LAST_RESULTS = None    # BassKernelResults of the most recent run


def kernel(query, key, attn_kernel, targets):
    global LAST_RESULTS
    query = np.asarray(query, dtype=np.float32)
    key = np.asarray(key, dtype=np.float32)
    attn_kernel = np.asarray(attn_kernel, dtype=np.float32)
    targets = np.asarray(targets, dtype=np.int32)

    _ensure_imports()
    from concourse.bass_utils import run_bass_kernel_spmd

    in_maps, node_order, n_win, n_slots, group_sizes = _host_arrays(
        query, key, attn_kernel, targets)
    nc = build_nc(n_slots, n_win, group_sizes)
    res = run_bass_kernel_spmd(
        nc, in_maps, list(range(N_CORES)),
        trace=TRACE, trace_cores=TRACE_CORES,
    )
    LAST_RESULTS = res

    # unscramble: psum partition-half x + column block b -> window w
    out = np.zeros((N_NODES, HC), dtype=np.float32)
    nbs = [(gs + 1) // 2 for gs in group_sizes]
    for c in range(N_CORES):
        oc = res.results[c]["out"]  # [64, nb_total*HC]
        bb = 0
        for gi, (gs, nb) in enumerate(zip(group_sizes, nbs)):
            for b in range(nb):
                for x in range(2):
                    w = gi * G + b * 2 + x
                    if w >= gi * G + gs:
                        continue
                    rows = node_order[c, w * WIN_NODES:(w + 1) * WIN_NODES]
                    valid = rows >= 0
                    blk = oc[x * WIN_NODES:(x + 1) * WIN_NODES,
                             (bb + b) * HC:(bb + b + 1) * HC]
                    out[rows[valid]] = blk[valid]
            bb += nb

    deg = np.bincount(targets, minlength=N_NODES)
    out[deg == 0] = 0.0
    return out


# revision 17
# speedup vs baseline: 1.1091x; 1.1091x over previous
"""GATv2 attention-pool kernel for 8 Trainium2 NeuronCores.

Algorithm
---------
Reference computes, per edge e with target node t(e):
    feats = q + k                                   [E, 64]
    logits[e,h] = sum_c feats[e,h*8+c] * A[c,h]     [E, 8]
    attn = segment_softmax(logits, targets)         [E, 8]
    out[n] = relu(segment_sum(q * attn))            [N, 64]

Because logits are O(20), exp() never overflows fp32/bf16, so the
segment-max shift is unnecessary and softmax folds into two segment-SUMS:
    denom[n,h]  = sum_{e->n} exp(logits[e,h])
    pooled[n,:] = sum_{e->n} q[e,:] * exp(logits[e,h])
    out[n]      = relu(pooled[n] / denom[n])

Distribution: edges are partitioned by target node (host-side sort), 100000
nodes split into 8 contiguous shards of 12500 -> all segment reductions are
core-local, no collectives.  Each shard is cut into windows of <= 32 nodes
whose edges are padded to TSUB*128 slots; G windows form a group brought in
by one contiguous DMA (12KB/partition descriptors).

Datapath is fp16 (bf16 for exp outputs, f32 psum accumulation):
 - q/k columns are host-permuted to (c,h) order so the per-head tree
   reduction and the ex-broadcast multiply keep a packed innermost dim.
 - the one-hot selector S is stored [node, subtile] so the rel broadcast
   lands on a middle dim; one tensor_tensor(is_equal) builds it per group.
 - per 128-edge subtile the PE accumulates psum += S^T @ [q*ex | ex]
   in fp16/bf16 (4x faster than fp32 matmul); two 32-node windows stack
   into the 64 psum partitions, then the epilogue divides, relus and
   permutes back to (h,c) once per node.

Host work is index metadata + data layout only (argsort of targets, gather
of q/k rows into slot order, fp16 packing); all floating-point arithmetic
runs on device.
"""

import os
import sys

import numpy as np

N_NODES = 100000
N_EDGES = 1600000
H = 8
C = 8
HC = H * C
MW = 2 * HC
N_CORES = 8
NODES_PER_CORE = N_NODES // N_CORES
WIN_NODES = 32
SUB = 128
TSUB = 4          # subtiles per window (window edge capacity = TSUB*128)
G = 12            # windows per group (one DMA + one DVE pass per group)
PJ = HC + H       # psum cols per window: 64 numerator + 8 denominator


def _ensure_imports():
    try:
        import concourse.bass  # noqa: F401
    except ImportError:
        for p in ("/opt/trn_rl_repo", "/root/.axon_site/_ro/trn_rl_repo"):
            if os.path.isdir(p) and p not in sys.path:
                sys.path.insert(0, p)


def pack_windows(targets):
    """Sort edges by target; bin-pack each core's nodes into windows.

    Every window holds at most WIN_NODES nodes AND at most TSUB*SUB edges
    (two-pointer big+small pairing keeps fragmentation low).
    """
    order = np.argsort(targets, kind="stable")
    tsorted = targets[order]
    node_start = np.searchsorted(tsorted, np.arange(N_NODES + 1))
    deg = np.diff(node_start)

    cap_e = TSUB * SUB
    packs = []
    for c in range(N_CORES):
        nodes = np.arange(c * NODES_PER_CORE, (c + 1) * NODES_PER_CORE)
        srt = sorted(deg[nodes][i] * 100000 + nodes[i] - c * NODES_PER_CORE
                     for i in range(NODES_PER_CORE))
        import bisect
        keys = [s // 100000 for s in srt]
        vals = [s % 100000 + c * NODES_PER_CORE for s in srt]
        wins = []
        while keys:
            # deficit steering: keep the window's running mean degree on
            # track for cap_e total over WIN_NODES nodes
            cur, cnt = [], 0
            while keys and len(cur) < WIN_NODES and cnt < cap_e:
                need = (cap_e - cnt) / (WIN_NODES - len(cur))
                i = bisect.bisect_left(keys, need)
                if i >= len(keys):
                    i = len(keys) - 1
                elif i > 0 and keys[i] > cap_e - cnt:
                    i -= 1
                if keys[i] > cap_e - cnt:
                    break
                cur.append(vals[i])
                cnt += keys[i]
                del keys[i], vals[i]
            wins.append(cur)
        packs.append(wins)
    return packs, order, node_start


def build_slots(packs, order, node_start):
    """Lay out edge slots in DRAM row order r = gbase + p*Tg + t.

    Window wi of a group owns subtile columns [wi*TSUB, (wi+1)*TSUB); within
    a window, edge j sits at (p = j // TSUB, t_local = j % TSUB), so each
    node's slot run is contiguous through the group-slab view.
    """
    n_win = max(len(w) for w in packs)
    n_groups = (n_win + G - 1) // G
    group_sizes = [min(G, n_win - gi * G) for gi in range(n_groups)]
    n_slots = sum(128 * gs * TSUB for gs in group_sizes)

    perms = np.zeros((N_CORES, n_slots), dtype=np.int64)
    rels = np.full((N_CORES, n_slots), -1.0, dtype=np.float16)
    node_order = np.full((N_CORES, n_win * WIN_NODES), -1, dtype=np.int64)
    for c in range(N_CORES):
        gbase = 0
        for gi, gs in enumerate(group_sizes):
            tg = gs * TSUB
            pslab = perms[c, gbase:gbase + 128 * tg].reshape(128, tg)
            rslab = rels[c, gbase:gbase + 128 * tg].reshape(128, tg)
            for wl in range(gs):
                w = gi * G + wl
                if w >= len(packs[c]):
                    continue
                pw = pslab[:, wl * TSUB:(wl + 1) * TSUB]
                rw = rslab[:, wl * TSUB:(wl + 1) * TSUB]
                pos = 0
                for j, node in enumerate(packs[c][w]):
                    e0, e1 = node_start[node], node_start[node + 1]
                    cnt = e1 - e0
                    # .flat writes through the non-contiguous column view
                    pw.flat[pos:pos + cnt] = order[e0:e1]
                    rw.flat[pos:pos + cnt] = j
                    pos += cnt
                    node_order[c, w * WIN_NODES + j] = node
            gbase += 128 * tg
    return perms, rels, node_order, n_win, n_slots, group_sizes


def build_nc(n_slots, n_win, group_sizes):
    """Build the single SPMD Bass program for one core's shard."""
    _ensure_imports()
    import concourse.bacc as bacc
    import concourse.mybir as mybir
    import concourse.tile as tile

    f32 = mybir.dt.float32
    f16 = mybir.dt.float16
    bf16 = mybir.dt.bfloat16

    OP = mybir.AluOpType
    AF = mybir.ActivationFunctionType

    tgmax = G * TSUB
    # two 32-node windows stack into 64 psum partitions; nb = column blocks
    nbs = [(gs + 1) // 2 for gs in group_sizes]
    nb_total = sum(nbs)

    nc = bacc.Bacc("TRN2", num_devices=N_CORES)
    qk = nc.declare_dram_parameter("qk", [n_slots, MW], f16, False)
    rel = nc.declare_dram_parameter("rel", [n_slots], f16, False)
    wrow = nc.declare_dram_parameter("wrow", [128, HC], f16, False)
    iof = nc.declare_dram_parameter("iof", [128, WIN_NODES * tgmax], f16,
                                    False)
    out = nc.declare_dram_parameter("out", [2 * WIN_NODES, nb_total * HC],
                                    f32, isOutput=True)

    with tile.TileContext(nc) as tc:
        with (
            tc.tile_pool(name="const", bufs=1) as cpool,
            tc.tile_pool(name="qk", bufs=4) as qkpool,
            tc.tile_pool(name="mid", bufs=2) as midpool,
            tc.tile_pool(name="lgp", bufs=4) as lgpool,
            tc.tile_pool(name="mm", bufs=4) as mmpool,
            tc.tile_pool(name="mt", bufs=4) as mtpool,
            tc.tile_pool(name="fin", bufs=3) as finpool,
            tc.tile_pool(name="psum", bufs=8, space="PSUM") as ppool,
        ):
            w_t = cpool.tile([128, HC], f16)
            nc.sync.dma_start(out=w_t[:], in_=wrow[:])
            io_t = cpool.tile([128, WIN_NODES, tgmax], f16)
            nc.sync.dma_start(
                out=io_t[:],
                in_=iof[:].rearrange("p (n t) -> p n t", t=tgmax))

            def emit_load(gi, gbase, gs):
                tg = gs * TSUB
                nsl = 128 * tg
                qk_t = qkpool.tile([128, tg, MW], f16, tag="qk")
                nc.sync.dma_start(
                    out=qk_t[:],
                    in_=qk[gbase:gbase + nsl, :].rearrange(
                        "(p t) c -> p t c", p=128),
                )
                r_t = qkpool.tile([128, tg], f16, tag="r")
                nc.sync.dma_start(
                    out=r_t[:],
                    in_=rel[gbase:gbase + nsl].rearrange(
                        "(p t) -> p t", p=128),
                )
                return {"gi": gi, "gs": gs, "tg": tg, "qk": qk_t, "r": r_t}

            def emit_logits(s):
                tg = s["tg"]
                qk3 = s["qk"]
                # f = q + k   (tensor_tensor is the fastest DVE op on hw)
                f_t = midpool.tile([128, tg, HC], f16, tag="f")
                nc.vector.tensor_tensor(
                    out=f_t[:], in0=qk3[:, :, 0:HC],
                    in1=qk3[:, :, HC:MW], op=OP.add)
                # wf = f * w  (w broadcast over t: middle dim, stays packed)
                wf_t = midpool.tile([128, tg, HC], f16, tag="wf")
                nc.vector.tensor_tensor(
                    out=wf_t[:], in0=f_t[:],
                    in1=w_t[:, None, :].to_broadcast([128, tg, HC]),
                    op=OP.mult)
                # tree-reduce over c (c-major layout: head h at col c*8+h)
                t1 = midpool.tile([128, tg, HC // 2], f16, tag="t1")
                nc.vector.tensor_tensor(
                    out=t1[:], in0=wf_t[:, :, 0:32],
                    in1=wf_t[:, :, 32:64], op=OP.add)
                t2 = midpool.tile([128, tg, HC // 4], f16, tag="t2")
                nc.vector.tensor_tensor(
                    out=t2[:], in0=t1[:, :, 0:16],
                    in1=t1[:, :, 16:32], op=OP.add)
                lg = lgpool.tile([128, tg, H], f16, tag="lg")
                nc.vector.tensor_tensor(
                    out=lg[:], in0=t2[:, :, 0:8],
                    in1=t2[:, :, 8:16], op=OP.add)
                # exp feeds the m multiply next iteration: issue it as soon
                # as the logits land so ACT runs it off the critical path
                m_t = mtpool.tile([128, tg, PJ], bf16, tag="M")
                nc.scalar.activation(
                    out=m_t[:, :, HC:PJ], in_=lg[:], func=AF.Exp)
                s["m"] = m_t

            def emit_spath(s):
                tg = s["tg"]
                # S[p, n, t] = (iota[n] == rel[p, t]) : rel broadcast on the
                # middle dim keeps the packed last dim
                s_t = mmpool.tile([128, WIN_NODES, tg], f16, tag="S")
                nc.vector.tensor_tensor(
                    out=s_t[:], in0=io_t[:, :, 0:tg],
                    in1=s["r"][:, None, :].to_broadcast(
                        [128, WIN_NODES, tg]),
                    op=OP.is_equal)
                s["S"] = s_t

            def emit_m(s):
                tg = s["tg"]
                m_t = s["m"]
                # m = q * ex (ex broadcast over c: middle dim, packed h last)
                nc.vector.tensor_tensor(
                    out=m_t[:, :, 0:HC].rearrange(
                        "p t (c h) -> p t c h", h=H),
                    in0=s["qk"][:, :, 0:HC].rearrange(
                        "p t (c h) -> p t c h", h=H),
                    in1=m_t[:, :, HC:PJ][:, :, None, :].to_broadcast(
                        [128, tg, C, H]),
                    op=OP.mult)
                s["m"] = m_t

            def emit_mm(s):
                gs = s["gs"]
                nb = (gs + 1) // 2
                p_t = ppool.tile([2 * WIN_NODES, nb * PJ], f32)
                for wl in range(gs):
                    x, b = wl % 2, wl // 2
                    prow = slice(x * WIN_NODES, (x + 1) * WIN_NODES)
                    pcols = slice(b * PJ, (b + 1) * PJ)
                    for g in range(TSUB):
                        t = wl * TSUB + g
                        nc.tensor.matmul(
                            p_t[prow, pcols],
                            lhsT=s["S"][:, :, t],
                            rhs=s["m"][:, t, :],
                            start=(g == 0),
                            stop=(g == TSUB - 1),
                        )
                s["psum"] = p_t

            def emit_epilogue(s, bbase):
                gs = s["gs"]
                nb = (gs + 1) // 2
                # drain psum once through the (idle) scalar engine; the
                # vector ops then read SBUF instead of slow PSUM
                pp = finpool.tile([2 * WIN_NODES, nb, PJ], f32, tag="pp")
                nc.scalar.activation(
                    out=pp[:], in_=s["psum"][:].rearrange(
                        "p (w j) -> p w j", j=PJ),
                    func=AF.Copy)
                rc_t = finpool.tile([2 * WIN_NODES, nb, H], f32, tag="rc")
                nc.vector.reciprocal(rc_t[:], pp[:, :, HC:PJ])
                # divide + permute numerator (c,h) -> (h,c)
                d_t = finpool.tile([2 * WIN_NODES, nb, H, C], f32, tag="d")
                nc.vector.tensor_tensor(
                    out=d_t[:].rearrange("p w h c -> p w c h"),
                    in0=pp[:, :, 0:HC].rearrange(
                        "p w (c h) -> p w c h", h=H),
                    in1=rc_t[:, :, None, :].to_broadcast(
                        [2 * WIN_NODES, nb, C, H]),
                    op=OP.mult)
                o_t = finpool.tile([2 * WIN_NODES, nb, HC], f32, tag="o")
                nc.scalar.activation(
                    o_t[:], d_t[:].rearrange("p w h c -> p w (h c)"),
                    func=AF.Relu)
                nc.sync.dma_start(
                    out=out[:, bbase * HC:(bbase + nb) * HC].rearrange(
                        "p (w c) -> p w c", c=HC),
                    in_=o_t[:],
                )

            gbases, bbases = [], []
            gb = bb = 0
            for gs, nb in zip(group_sizes, nbs):
                gbases.append(gb)
                bbases.append(bb)
                gb += 128 * gs * TSUB
                bb += nb

            ng = len(group_sizes)

            def load(i):
                return emit_load(i, gbases[i], group_sizes[i])

            # software pipeline with 2-group lookahead: the epilogue of
            # group i is emitted two iterations after its matmuls, so the
            # DVE always has group i+2's logits/S work while the PE
            # finishes group i+1
            st = [None] * ng
            st[0] = load(0)
            emit_logits(st[0])
            emit_spath(st[0])
            if ng > 1:
                st[1] = load(1)
            for i in range(ng):
                if i + 2 < ng:
                    st[i + 2] = load(i + 2)
                emit_m(st[i])
                emit_mm(st[i])
                if i + 1 < ng:
                    emit_logits(st[i + 1])
                    emit_spath(st[i + 1])
                if i >= 1:
                    emit_epilogue(st[i - 1], bbases[i - 1])
                    st[i - 1] = None
            emit_epilogue(st[ng - 1], bbases[ng - 1])

    nc.finalize()
    return nc


def _host_arrays(query, key, attn_kernel, targets):
    packs, order, node_start = pack_windows(targets)
    perms, rels, node_order, n_win, n_slots, group_sizes = build_slots(
        packs, order, node_start)

    # (c,h)-major column permutation: new col c*8+h <- orig col h*8+c
    colperm = (np.arange(HC).reshape(C, H).T).reshape(-1)  # [c*8+h] = h*8+c
    wrow_1 = np.asarray(attn_kernel, dtype=np.float16).reshape(-1)  # A[c,h]
    wrow = np.tile(wrow_1, (128, 1))

    tgmax = G * TSUB
    iof = np.tile(
        np.repeat(np.arange(WIN_NODES, dtype=np.float16), tgmax), (128, 1))

    q16 = query[:, colperm].astype(np.float16)
    k16 = key[:, colperm].astype(np.float16)
    in_maps = []
    for c in range(N_CORES):
        qkc = np.zeros((n_slots, MW), dtype=np.float16)
        pc = perms[c]
        used = rels[c] >= 0
        qkc[used, :HC] = q16[pc[used]]
        qkc[used, HC:] = k16[pc[used]]
        in_maps.append({
            "qk": qkc,
            "rel": rels[c],
            "wrow": wrow,
            "iof": iof,
        })
    return in_maps, node_order, n_win, n_slots, group_sizes


TRACE = False          # set by test harness to capture an NTFF profile
TRACE_CORES = None
LAST_RESULTS = None    # BassKernelResults of the most recent run


def kernel(query, key, attn_kernel, targets):
    global LAST_RESULTS
    query = np.asarray(query, dtype=np.float32)
    key = np.asarray(key, dtype=np.float32)
    attn_kernel = np.asarray(attn_kernel, dtype=np.float32)
    targets = np.asarray(targets, dtype=np.int32)

    _ensure_imports()
    from concourse.bass_utils import run_bass_kernel_spmd

    in_maps, node_order, n_win, n_slots, group_sizes = _host_arrays(
        query, key, attn_kernel, targets)
    nc = build_nc(n_slots, n_win, group_sizes)
    res = run_bass_kernel_spmd(
        nc, in_maps, list(range(N_CORES)),
        trace=TRACE, trace_cores=TRACE_CORES,
    )
    LAST_RESULTS = res

    # unscramble: psum partition-half x + column block b -> window w
    out = np.zeros((N_NODES, HC), dtype=np.float32)
    nbs = [(gs + 1) // 2 for gs in group_sizes]
    for c in range(N_CORES):
        oc = res.results[c]["out"]  # [64, nb_total*HC]
        bb = 0
        for gi, (gs, nb) in enumerate(zip(group_sizes, nbs)):
            for b in range(nb):
                for x in range(2):
                    w = gi * G + b * 2 + x
                    if w >= gi * G + gs:
                        continue
                    rows = node_order[c, w * WIN_NODES:(w + 1) * WIN_NODES]
                    valid = rows >= 0
                    blk = oc[x * WIN_NODES:(x + 1) * WIN_NODES,
                             (bb + b) * HC:(bb + b + 1) * HC]
                    out[rows[valid]] = blk[valid]
            bb += nb

    deg = np.bincount(targets, minlength=N_NODES)
    out[deg == 0] = 0.0
    return out


# revision 20
# speedup vs baseline: 1.3209x; 1.1910x over previous
"""GATv2 attention-pool kernel for 8 Trainium2 NeuronCores.

Algorithm
---------
Reference computes, per edge e with target node t(e):
    feats = q + k                                   [E, 64]
    logits[e,h] = sum_c feats[e,h*8+c] * A[c,h]     [E, 8]
    attn = segment_softmax(logits, targets)         [E, 8]
    out[n] = relu(segment_sum(q * attn))            [N, 64]

Because logits are O(20), exp() never overflows fp32/bf16, so the
segment-max shift is unnecessary and softmax folds into two segment-SUMS:
    denom[n,h]  = sum_{e->n} exp(logits[e,h])
    pooled[n,:] = sum_{e->n} q[e,:] * exp(logits[e,h])
    out[n]      = relu(pooled[n] / denom[n])

Distribution: edges are partitioned by target node (host-side sort), 100000
nodes split into 8 contiguous shards of 12500 -> all segment reductions are
core-local, no collectives.  Each shard is cut into windows of <= 32 nodes
whose edges are padded to TSUB*128 slots; G windows form a group brought in
by one contiguous DMA (12KB/partition descriptors).

Datapath is fp16 (bf16 for exp outputs, f32 psum accumulation):
 - q/k columns are host-permuted to (c,h) order so the per-head tree
   reduction and the ex-broadcast multiply keep a packed innermost dim.
 - the one-hot selector S is stored [node, subtile] so the rel broadcast
   lands on a middle dim; one tensor_tensor(is_equal) builds it per group.
 - per 128-edge subtile the PE accumulates psum += S^T @ [q*ex | ex]
   in fp16/bf16 (4x faster than fp32 matmul); two 32-node windows stack
   into the 64 psum partitions, then the epilogue divides, relus and
   permutes back to (h,c) once per node.

Host work is index metadata + data layout only (argsort of targets, gather
of q/k rows into slot order, fp16 packing); all floating-point arithmetic
runs on device.
"""

import os
import sys

import numpy as np

N_NODES = 100000
N_EDGES = 1600000
H = 8
C = 8
HC = H * C
MW = 2 * HC
N_CORES = 8
NODES_PER_CORE = N_NODES // N_CORES
WIN_NODES = 32
SUB = 128
TSUB = 4          # subtiles per window (window edge capacity = TSUB*128)
G = 12            # windows per group (one DMA + one DVE pass per group)
PJ = HC + H       # psum cols per window: 64 numerator + 8 denominator


def _ensure_imports():
    try:
        import concourse.bass  # noqa: F401
    except ImportError:
        for p in ("/opt/trn_rl_repo", "/root/.axon_site/_ro/trn_rl_repo"):
            if os.path.isdir(p) and p not in sys.path:
                sys.path.insert(0, p)


def pack_windows(targets):
    """Sort edges by target; bin-pack each core's nodes into windows.

    Every window holds at most WIN_NODES nodes AND at most TSUB*SUB edges
    (two-pointer big+small pairing keeps fragmentation low).
    """
    order = np.argsort(targets, kind="stable")
    tsorted = targets[order]
    node_start = np.searchsorted(tsorted, np.arange(N_NODES + 1))
    deg = np.diff(node_start)

    cap_e = TSUB * SUB
    packs = []
    for c in range(N_CORES):
        nodes = np.arange(c * NODES_PER_CORE, (c + 1) * NODES_PER_CORE)
        srt = sorted(deg[nodes][i] * 100000 + nodes[i] - c * NODES_PER_CORE
                     for i in range(NODES_PER_CORE))
        import bisect
        keys = [s // 100000 for s in srt]
        vals = [s % 100000 + c * NODES_PER_CORE for s in srt]
        wins = []
        while keys:
            # deficit steering: keep the window's running mean degree on
            # track for cap_e total over WIN_NODES nodes
            cur, cnt = [], 0
            while keys and len(cur) < WIN_NODES and cnt < cap_e:
                need = (cap_e - cnt) / (WIN_NODES - len(cur))
                i = bisect.bisect_left(keys, need)
                if i >= len(keys):
                    i = len(keys) - 1
                elif i > 0 and keys[i] > cap_e - cnt:
                    i -= 1
                if keys[i] > cap_e - cnt:
                    break
                cur.append(vals[i])
                cnt += keys[i]
                del keys[i], vals[i]
            wins.append(cur)
        packs.append(wins)
    return packs, order, node_start


def build_slots(packs, order, node_start):
    """Lay out edge slots in DRAM row order r = gbase + p*Tg + t.

    Window wi of a group owns subtile columns [wi*TSUB, (wi+1)*TSUB); within
    a window, edge j sits at (p = j // TSUB, t_local = j % TSUB), so each
    node's slot run is contiguous through the group-slab view.
    """
    n_win = max(len(w) for w in packs)
    n_groups = (n_win + G - 1) // G
    group_sizes = [min(G, n_win - gi * G) for gi in range(n_groups)]
    n_slots = sum(128 * gs * TSUB for gs in group_sizes)

    perms = np.zeros((N_CORES, n_slots), dtype=np.int64)
    rels = np.full((N_CORES, n_slots), -1.0, dtype=np.float16)
    node_order = np.full((N_CORES, n_win * WIN_NODES), -1, dtype=np.int64)
    for c in range(N_CORES):
        gbase = 0
        for gi, gs in enumerate(group_sizes):
            tg = gs * TSUB
            pslab = perms[c, gbase:gbase + 128 * tg].reshape(128, tg)
            rslab = rels[c, gbase:gbase + 128 * tg].reshape(128, tg)
            for wl in range(gs):
                w = gi * G + wl
                if w >= len(packs[c]):
                    continue
                pw = pslab[:, wl * TSUB:(wl + 1) * TSUB]
                rw = rslab[:, wl * TSUB:(wl + 1) * TSUB]
                pos = 0
                for j, node in enumerate(packs[c][w]):
                    e0, e1 = node_start[node], node_start[node + 1]
                    cnt = e1 - e0
                    # .flat writes through the non-contiguous column view
                    pw.flat[pos:pos + cnt] = order[e0:e1]
                    rw.flat[pos:pos + cnt] = j
                    pos += cnt
                    node_order[c, w * WIN_NODES + j] = node
            gbase += 128 * tg
    return perms, rels, node_order, n_win, n_slots, group_sizes


def build_nc(n_slots, n_win, group_sizes):
    """Build the single SPMD Bass program for one core's shard."""
    _ensure_imports()
    import concourse.bacc as bacc
    import concourse.mybir as mybir
    import concourse.tile as tile

    f32 = mybir.dt.float32
    f16 = mybir.dt.float16
    bf16 = mybir.dt.bfloat16

    OP = mybir.AluOpType
    AF = mybir.ActivationFunctionType

    tgmax = G * TSUB
    # two 32-node windows stack into 64 psum partitions; nb = column blocks
    nbs = [(gs + 1) // 2 for gs in group_sizes]
    nb_total = sum(nbs)

    nc = bacc.Bacc("TRN2", num_devices=N_CORES)
    qk = nc.declare_dram_parameter("qk", [n_slots, MW], f16, False)
    rel = nc.declare_dram_parameter("rel", [n_slots], f16, False)
    wrow = nc.declare_dram_parameter("wrow", [128, HC], f16, False)
    iof = nc.declare_dram_parameter("iof", [128, WIN_NODES * tgmax], f16,
                                    False)
    out = nc.declare_dram_parameter("out", [2 * WIN_NODES, nb_total * HC],
                                    f32, isOutput=True)

    with tile.TileContext(nc) as tc:
        with (
            tc.tile_pool(name="const", bufs=1) as cpool,
            tc.tile_pool(name="qk", bufs=4) as qkpool,
            tc.tile_pool(name="mid", bufs=2) as midpool,
            tc.tile_pool(name="lgp", bufs=4) as lgpool,
            tc.tile_pool(name="mm", bufs=4) as mmpool,
            tc.tile_pool(name="mt", bufs=3) as mtpool,
            tc.tile_pool(name="fin", bufs=3) as finpool,
            tc.tile_pool(name="psum", bufs=8, space="PSUM") as ppool,
        ):
            w_t = cpool.tile([128, HC], f16)
            nc.sync.dma_start(out=w_t[:], in_=wrow[:])
            io_t = cpool.tile([128, WIN_NODES, tgmax], f16)
            nc.sync.dma_start(
                out=io_t[:],
                in_=iof[:].rearrange("p (n t) -> p n t", t=tgmax))

            def emit_load(gi, gbase, gs):
                tg = gs * TSUB
                nsl = 128 * tg
                qk_t = qkpool.tile([128, tg, MW], f16, tag="qk")
                nc.sync.dma_start(
                    out=qk_t[:],
                    in_=qk[gbase:gbase + nsl, :].rearrange(
                        "(p t) c -> p t c", p=128),
                )
                r_t = qkpool.tile([128, tg], f16, tag="r")
                nc.sync.dma_start(
                    out=r_t[:],
                    in_=rel[gbase:gbase + nsl].rearrange(
                        "(p t) -> p t", p=128),
                )
                return {"gi": gi, "gs": gs, "tg": tg, "qk": qk_t, "r": r_t}

            def emit_logits(s):
                tg = s["tg"]
                qk3 = s["qk"]
                # f = q + k   (tensor_tensor is the fastest DVE op on hw)
                f_t = midpool.tile([128, tg, HC], f16, tag="f")
                nc.vector.tensor_tensor(
                    out=f_t[:], in0=qk3[:, :, 0:HC],
                    in1=qk3[:, :, HC:MW], op=OP.add)
                # wf = f * w  (w broadcast over t: middle dim, stays packed)
                wf_t = midpool.tile([128, tg, HC], f16, tag="wf")
                nc.vector.tensor_tensor(
                    out=wf_t[:], in0=f_t[:],
                    in1=w_t[:, None, :].to_broadcast([128, tg, HC]),
                    op=OP.mult)
                # tree-reduce over c (c-major layout: head h at col c*8+h)
                t1 = midpool.tile([128, tg, HC // 2], f16, tag="t1")
                nc.vector.tensor_tensor(
                    out=t1[:], in0=wf_t[:, :, 0:32],
                    in1=wf_t[:, :, 32:64], op=OP.add)
                t2 = midpool.tile([128, tg, HC // 4], f16, tag="t2")
                nc.vector.tensor_tensor(
                    out=t2[:], in0=t1[:, :, 0:16],
                    in1=t1[:, :, 16:32], op=OP.add)
                lg = lgpool.tile([128, tg, H], f16, tag="lg")
                nc.vector.tensor_tensor(
                    out=lg[:], in0=t2[:, :, 0:8],
                    in1=t2[:, :, 8:16], op=OP.add)
                s["lg"] = lg

            def emit_spath(s):
                tg = s["tg"]
                # S[p, n, t] = (iota[n] == rel[p, t]) : rel broadcast on the
                # middle dim keeps the packed last dim
                s_t = mmpool.tile([128, WIN_NODES, tg], f16, tag="S")
                nc.vector.tensor_tensor(
                    out=s_t[:], in0=io_t[:, :, 0:tg],
                    in1=s["r"][:, None, :].to_broadcast(
                        [128, WIN_NODES, tg]),
                    op=OP.is_equal)
                s["S"] = s_t

            def emit_m(s):
                tg = s["tg"]
                m_t = mtpool.tile([128, tg, PJ], bf16, tag="M")
                nc.scalar.activation(
                    out=m_t[:, :, HC:PJ], in_=s["lg"][:], func=AF.Exp)
                # m = q * ex (ex broadcast over c: middle dim, packed h last)
                nc.vector.tensor_tensor(
                    out=m_t[:, :, 0:HC].rearrange(
                        "p t (c h) -> p t c h", h=H),
                    in0=s["qk"][:, :, 0:HC].rearrange(
                        "p t (c h) -> p t c h", h=H),
                    in1=m_t[:, :, HC:PJ][:, :, None, :].to_broadcast(
                        [128, tg, C, H]),
                    op=OP.mult)
                s["m"] = m_t

            def emit_mm(s):
                gs = s["gs"]
                nb = (gs + 1) // 2
                p_t = ppool.tile([2 * WIN_NODES, nb * PJ], f32)
                for wl in range(gs):
                    x, b = wl % 2, wl // 2
                    prow = slice(x * WIN_NODES, (x + 1) * WIN_NODES)
                    pcols = slice(b * PJ, (b + 1) * PJ)
                    for g in range(TSUB):
                        t = wl * TSUB + g
                        nc.tensor.matmul(
                            p_t[prow, pcols],
                            lhsT=s["S"][:, :, t],
                            rhs=s["m"][:, t, :],
                            start=(g == 0),
                            stop=(g == TSUB - 1),
                        )
                s["psum"] = p_t

            def emit_epilogue(s, bbase):
                gs = s["gs"]
                nb = (gs + 1) // 2
                # drain psum once through the (idle) scalar engine; the
                # vector ops then read SBUF instead of slow PSUM
                pp = finpool.tile([2 * WIN_NODES, nb, PJ], f32, tag="pp")
                nc.scalar.activation(
                    out=pp[:], in_=s["psum"][:].rearrange(
                        "p (w j) -> p w j", j=PJ),
                    func=AF.Copy)
                rc_t = finpool.tile([2 * WIN_NODES, nb, H], f32, tag="rc")
                nc.vector.reciprocal(rc_t[:], pp[:, :, HC:PJ])
                # divide + permute numerator (c,h) -> (h,c)
                d_t = finpool.tile([2 * WIN_NODES, nb, H, C], f32, tag="d")
                nc.vector.tensor_tensor(
                    out=d_t[:].rearrange("p w h c -> p w c h"),
                    in0=pp[:, :, 0:HC].rearrange(
                        "p w (c h) -> p w c h", h=H),
                    in1=rc_t[:, :, None, :].to_broadcast(
                        [2 * WIN_NODES, nb, C, H]),
                    op=OP.mult)
                o_t = finpool.tile([2 * WIN_NODES, nb, HC], f32, tag="o")
                nc.scalar.activation(
                    o_t[:], d_t[:].rearrange("p w h c -> p w (h c)"),
                    func=AF.Relu)
                nc.sync.dma_start(
                    out=out[:, bbase * HC:(bbase + nb) * HC].rearrange(
                        "p (w c) -> p w c", c=HC),
                    in_=o_t[:],
                )

            gbases, bbases = [], []
            gb = bb = 0
            for gs, nb in zip(group_sizes, nbs):
                gbases.append(gb)
                bbases.append(bb)
                gb += 128 * gs * TSUB
                bb += nb

            ng = len(group_sizes)

            def load(i):
                return emit_load(i, gbases[i], group_sizes[i])

            # software pipeline with 2-group lookahead: the epilogue of
            # group i is emitted two iterations after its matmuls, so the
            # DVE always has group i+2's logits/S work while the PE
            # finishes group i+1
            st = [None] * ng
            st[0] = load(0)
            emit_logits(st[0])
            emit_spath(st[0])
            if ng > 1:
                st[1] = load(1)
            for i in range(ng):
                if i + 2 < ng:
                    st[i + 2] = load(i + 2)
                emit_m(st[i])
                emit_mm(st[i])
                if i + 1 < ng:
                    emit_logits(st[i + 1])
                    emit_spath(st[i + 1])
                if i >= 1:
                    emit_epilogue(st[i - 1], bbases[i - 1])
                    st[i - 1] = None
            emit_epilogue(st[ng - 1], bbases[ng - 1])

    nc.finalize()
    return nc


def _host_arrays(query, key, attn_kernel, targets):
    packs, order, node_start = pack_windows(targets)
    perms, rels, node_order, n_win, n_slots, group_sizes = build_slots(
        packs, order, node_start)

    # (c,h)-major column permutation: new col c*8+h <- orig col h*8+c
    colperm = (np.arange(HC).reshape(C, H).T).reshape(-1)  # [c*8+h] = h*8+c
    wrow_1 = np.asarray(attn_kernel, dtype=np.float16).reshape(-1)  # A[c,h]
    wrow = np.tile(wrow_1, (128, 1))

    tgmax = G * TSUB
    iof = np.tile(
        np.repeat(np.arange(WIN_NODES, dtype=np.float16), tgmax), (128, 1))

    q16 = query[:, colperm].astype(np.float16)
    k16 = key[:, colperm].astype(np.float16)
    in_maps = []
    for c in range(N_CORES):
        qkc = np.zeros((n_slots, MW), dtype=np.float16)
        pc = perms[c]
        used = rels[c] >= 0
        qkc[used, :HC] = q16[pc[used]]
        qkc[used, HC:] = k16[pc[used]]
        in_maps.append({
            "qk": qkc,
            "rel": rels[c],
            "wrow": wrow,
            "iof": iof,
        })
    return in_maps, node_order, n_win, n_slots, group_sizes


TRACE = False          # set by test harness to capture an NTFF profile
TRACE_CORES = None
LAST_RESULTS = None    # BassKernelResults of the most recent run


def kernel(query, key, attn_kernel, targets):
    global LAST_RESULTS
    query = np.asarray(query, dtype=np.float32)
    key = np.asarray(key, dtype=np.float32)
    attn_kernel = np.asarray(attn_kernel, dtype=np.float32)
    targets = np.asarray(targets, dtype=np.int32)

    _ensure_imports()
    from concourse.bass_utils import run_bass_kernel_spmd

    in_maps, node_order, n_win, n_slots, group_sizes = _host_arrays(
        query, key, attn_kernel, targets)
    nc = build_nc(n_slots, n_win, group_sizes)
    res = run_bass_kernel_spmd(
        nc, in_maps, list(range(N_CORES)),
        trace=TRACE, trace_cores=TRACE_CORES,
    )
    LAST_RESULTS = res

    # unscramble: psum partition-half x + column block b -> window w
    out = np.zeros((N_NODES, HC), dtype=np.float32)
    nbs = [(gs + 1) // 2 for gs in group_sizes]
    for c in range(N_CORES):
        oc = res.results[c]["out"]  # [64, nb_total*HC]
        bb = 0
        for gi, (gs, nb) in enumerate(zip(group_sizes, nbs)):
            for b in range(nb):
                for x in range(2):
                    w = gi * G + b * 2 + x
                    if w >= gi * G + gs:
                        continue
                    rows = node_order[c, w * WIN_NODES:(w + 1) * WIN_NODES]
                    valid = rows >= 0
                    blk = oc[x * WIN_NODES:(x + 1) * WIN_NODES,
                             (bb + b) * HC:(bb + b + 1) * HC]
                    out[rows[valid]] = blk[valid]
            bb += nb

    deg = np.bincount(targets, minlength=N_NODES)
    out[deg == 0] = 0.0
    return out
